# revision 11
# baseline (speedup 1.0000x reference)
"""Masked multi-head attention (B=2, H=16, S=2048, D=64) on 8 TRN2 NeuronCores.

Sharding: batch*heads (32) split 4-heads-per-core across 8 cores; each core
computes full attention for its heads; the boolean mask is shared (broadcast
to every core). No cross-device communication.

Device algorithm (per head), in transposed "S^T" layout so softmax
probabilities land with the contraction (k) dim on partitions and feed the
P@V matmul with no on-device transposes:

  S^T[k, q]  = K^T[d, k].T @ Q^T[d, q]          (PE, d=64 contraction,
                                                 consecutive k-blocks packed
                                                 in opposite PE row halves)
  P^T[k, q]  = exp(scale * S^T) * keepT[k, q]
  O^T_aug    = V_aug[k, d+1].T @ P^T[k, q]      (PE, accumulated over k in
                                                 PSUM; V_aug has a ones
                                                 column -> row d holds the
                                                 softmax denominators)

The exp work is split across three engines to break the ACT bottleneck
(baseline: all 128 exps/core on ACT = ~140us busy vs ~110us PE):
  * ACT k-block pairs: exact spline exp (ACT), then keep-mult on DVE/GPSIMD.
  * DVE k-block pairs: 2-instruction Schraudolph in fp16 bit domain:
      op1 (DVE tensor_scalar): bits_i16 = int16(EA*s + EB)   [trunc+wrap;
          values stay in-range by construction, |s| <= ~60]
      op2 (custom DVE op):     p = bits.f16 * ((XC2*m + XC1)*m + XC0)
          where m = 1+frac extracted with BITWISE_AND/OR on the fp32
          bit pattern; max rel err 0.64% (vs 3% uncorrected sawtooth).
    then the same keep-mult.
The keep multiplies and the PSUM->SBUF output copies are distributed over
DVE / GPSIMD / ACT so every engine sits just under the PE roofline.

O^T_aug [65, 2048] fp32 is copied PSUM->SBUF and DMA'd to HBM; the final
normalize (divide by denominator row) + un-transpose happens on host.

Softmax needs no running-max: scores*scale ~ N(0,1), |max| < ~7, exp() is
safely in fp32/fp16-bitdomain range, and softmax is shift-invariant.
Masked entries match the reference exactly: reference adds -1e4 so exp
underflows to 0.0 in fp32; we multiply by keep=0.
"""

import sys

for _p in ("/opt/trn_rl_repo", "/root/.axon_site/_ro/trn_rl_repo"):
    if _p not in sys.path:
        sys.path.append(_p)

import numpy as np
import ml_dtypes

B, H, S, D = 2, 16, 2048, 64
N_CORES = 8
HPC = (B * H) // N_CORES  # heads per core
P = 128
KB = S // P               # k blocks per head
QH = 2                    # q halves (exp tile free dim = S/QH)
QW = S // QH
SCALE = 1.0 / 8.0         # 1/sqrt(D)

QK_DTYPE = "float32r"

# Schraudolph fp16-bitdomain exp constants (op1) + quadratic mantissa
# correction (op2).  Fit numerically for trunc() conversion, EB=15360:
# max rel err 0.64% on x in [-8, 8].
EA = float(1024.0 * SCALE / np.log(2.0))
EB = 15360.0          # for op2-corrected pairs
EB_UNC = 15312.0      # minimax-centered for uncorrected pairs (+-3%)
XC2, XC1, XC0 = 0.2263466, -0.66686962, 1.4347952
MBITSF = float(np.int32(0x007FE000).view(np.float32))

# --- engine assignment schedule -------------------------------------------
# Per (h, qh) iteration i = h*QH + qh (8 per core), per k-block-pair kbp
# (8 per iteration):
#   DVE_PAIRS[i % len]: kbps whose exp runs on DVE (op1 [+op2]); others ACT.
#   FIX_PAIRS[i % len]: DVE kbps that also get the op2 mantissa correction.
#   MASK_GPS[i % len]:  kbps whose keep-mult runs on GPSIMD (else DVE).
#   OB_ENGINE[i % len]: engine for the [65, QW] PSUM->SBUF output copy
#                       ("act" / "dve"; GPSIMD has no PSUM port).
# Measured HW per-op costs ([128, 2048] tile-pair): ACT exp-pair ~2.4us,
# DVE op1 ~2.5us, DVE op2 (custom) ~2.2-4.5us, DVE keep-mult ~1.1us,
# GPSIMD keep-mult ~5.4us, ob copy ~1.2us. PE ~112-118us total.
SCHEDULES = {
    # name: (DVE_PAIRS, FIX_PAIRS, MASK_GPS, OB_ENGINE)
    "v3": (
        (frozenset({2, 5}), frozenset({1, 6})),
        (frozenset(), frozenset()),
        (frozenset({3}), frozenset({7})),
        ("dve", "act"),
    ),
    "v3fix": (
        (frozenset({2, 5}), frozenset({1, 6})),
        (frozenset({2}), frozenset({6})),
        (frozenset({3}), frozenset({7})),
        ("dve", "act"),
    ),
    "v3allfix": (
        (frozenset({2, 5}), frozenset({1, 6})),
        (frozenset({2, 5}), frozenset({1, 6})),
        (frozenset({3}), frozenset({7})),
        ("dve", "act"),
    ),
    "v3g0": (
        (frozenset({2, 5}), frozenset({1, 6})),
        (frozenset(), frozenset()),
        (frozenset(), frozenset()),
        ("dve", "act"),
    ),
    "all_act": (
        (frozenset(), frozenset()),
        (frozenset(), frozenset()),
        (frozenset({3}), frozenset({7})),
        ("dve", "act"),
    ),
}

DEFAULT_OPTS = ("v3fix",)

_CACHE = {}


def _register_schraud_fix():
    """Runtime-register the custom DVE mantissa-correction op.

    out = in0 * ((s1*m + imm2)*m + c3),  m = or(and(bits(in0), s0), 1.0)
    (s0 = fp32 mantissa mask 0x007FE000 as float; c3 via in1 latch).
    """
    from concourse import dve_ops
    from concourse.dve_spec import (
        Spec, Src0, C0, C1, C2, C3, One, Bin, AluOp, lower, _spill_c3_to_src1,
    )
    from concourse.dve_uop import DveOpSpec

    name = "SCHRAUD_FIX_ANT"
    for o in dve_ops.OPS:
        if o.name == name:
            return o

    m = Bin(AluOp.BITWISE_OR, Bin(AluOp.BITWISE_AND, Src0, C0), One)
    g = (C1 * m + C2) * m + C3
    body = _spill_c3_to_src1(Src0 * g)

    def ref(in0, in1, s0, s1, imm2):
        x32 = in0.astype(np.float32)
        bits = x32.view(np.uint32)
        maskb = np.float32(
            s0 if np.isscalar(s0) else np.asarray(s0).ravel()[0]
        ).view(np.uint32)
        mbits = (bits & maskb) | np.float32(1.0).view(np.uint32)
        mm = mbits.view(np.float32)
        c3 = np.float32(np.asarray(in1).ravel()[0])
        gg = (np.float32(s1) * mm + np.float32(imm2)) * mm + c3
        return x32 * gg

    op = dve_ops.DveOp(name, Spec(body=body, reference=ref), subdim=False,
                       uops_sha={})
    dve_ops.OPS.append(op)
    dve_ops.CUSTOM_DVE_SPECS[name] = op.spec
    dve_ops._SUB_OPCODE_FOR_NAME[name] = (
        max(dve_ops._SUB_OPCODE_FOR_NAME.values()) + 1
    )
    assert dve_ops._SUB_OPCODE_FOR_NAME[name] < 0x20
    from concourse.dve_ops import get_dve_sub_opcode

    for ver in ("v3", "v4"):
        uops = lower(op.spec, ver=ver)
        tmp = DveOpSpec(name=name, opcode=get_dve_sub_opcode(name), uops=uops,
                        rd1_en=True)
        op.uops_sha[ver] = tmp.sha(ver)
    return op


def _build(repeats=1, qk_dtype=QK_DTYPE, opts=()):
    opts = frozenset(opts)
    import concourse.mybir as mybir
    import concourse.tile as tile
    from concourse import bacc

    fix_op = _register_schraud_fix()

    dt = mybir.dt
    qk_dt = getattr(dt, qk_dtype)
    half_dt = dt.float16
    nc = bacc.Bacc(
        "TRN2", target_bir_lowering=False, debug=False, num_devices=N_CORES
    )

    qT = nc.dram_tensor("qT", [HPC, D, S], qk_dt, kind="ExternalInput").ap()
    kT = nc.dram_tensor("kT", [HPC, D, S], qk_dt, kind="ExternalInput").ap()
    v = nc.dram_tensor("v", [HPC, S, D], half_dt, kind="ExternalInput").ap()
    keepT = nc.dram_tensor("keepT", [S, S], half_dt, kind="ExternalInput").ap()
    out = nc.dram_tensor(
        "out", [HPC, D + 1, S], dt.float32, kind="ExternalOutput"
    ).ap()

    Exp = mybir.ActivationFunctionType.Exp
    mult = mybir.AluOpType.mult
    add = mybir.AluOpType.add

    sched_name = next((o for o in opts if o in SCHEDULES), "v3fix")
    DVE_PAIRS, FIX_PAIRS, MASK_GPS, OB_ENGINE = SCHEDULES[sched_name]

    with tile.TileContext(nc) as tc:
        with (
            tc.tile_pool(name="keep_pool", bufs=1) as keep_pool,
            tc.tile_pool(name="const_pool", bufs=1) as const_pool,
            tc.tile_pool(name="qk_pool", bufs=2) as qk_pool,
            tc.tile_pool(name="v_pool", bufs=2) as v_pool,
            tc.tile_pool(name="p_pool", bufs=3) as p_pool,
            tc.tile_pool(name="ob_pool", bufs=2) as ob_pool,
            tc.tile_pool(name="s_psum", bufs=3, space="PSUM") as s_psum,
            tc.tile_pool(name="o_psum", bufs=1, space="PSUM") as o_psum,
        ):
            keep_sb = keep_pool.tile([P, KB, S], half_dt)
            for kb in range(KB):
                nc.sync.dma_start(
                    out=keep_sb[:, kb, :], in_=keepT[kb * P:(kb + 1) * P, :]
                )
            c3_sb = const_pool.tile([P, 1], dt.float32)
            nc.gpsimd.memset(c3_sb[:, :], XC0)

            def body(rep):
                for h in range(HPC):
                    # Q^T/K^T [64, S] duplicated into both partition halves
                    # so consecutive k-blocks run in opposite PE row halves.
                    qTr = qk_pool.tile([P, S], qk_dt, tag="qTr", name=f"qTr_{h}")
                    kTr = qk_pool.tile([P, S], qk_dt, tag="kTr", name=f"kTr_{h}")
                    for half in (0, 1):
                        nc.sync.dma_start(
                            out=qTr[half * 64:(half + 1) * 64, :], in_=qT[h]
                        )
                        nc.sync.dma_start(
                            out=kTr[half * 64:(half + 1) * 64, :], in_=kT[h]
                        )

                    # V with an appended ones column (denominator column).
                    v_sb = v_pool.tile(
                        [P, KB, D + 1], half_dt, tag="v", name=f"v_{h}"
                    )
                    v_re = v[h].rearrange("(kb p) d -> p kb d", p=P)
                    nc.sync.dma_start(out=v_sb[:, :, 0:D], in_=v_re)
                    nc.gpsimd.memset(v_sb[:, :, D:D + 1], 1.0)

                    for qh in range(QH):
                        it = h * QH + qh
                        dve_pairs = DVE_PAIRS[it % len(DVE_PAIRS)]
                        fix_pairs = FIX_PAIRS[it % len(FIX_PAIRS)]
                        mask_gps = MASK_GPS[it % len(MASK_GPS)]
                        ob_eng = OB_ENGINE[it % len(OB_ENGINE)]

                        ot = o_psum.tile(
                            [D + 1, QW], dt.float32, tag="ot",
                            name=f"ot_{h}_{qh}",
                        )
                        for kbp in range(KB // 2):
                            use_dve = kbp in dve_pairs
                            use_fix = use_dve and kbp in fix_pairs
                            if not (use_dve and not use_fix):
                                p2 = p_pool.tile(
                                    [P, 2, QW], half_dt, tag="p",
                                    name=f"p_{h}_{qh}_{kbp}",
                                )
                            if use_dve:
                                bits2 = p_pool.tile(
                                    [P, 2, QW], dt.int16, tag="bits",
                                    name=f"bits_{h}_{qh}_{kbp}",
                                )
                            for e in (0, 1):
                                kb = 2 * kbp + e
                                half = 64 * e
                                s_ps = s_psum.tile(
                                    [P, QW], dt.float32, tag="s",
                                    name=f"s_{h}_{qh}_{kb}",
                                )
                                for qc in range(QW // 512):
                                    q0 = qh * QW + qc * 512
                                    nc.tensor.matmul(
                                        s_ps[:, qc * 512:(qc + 1) * 512],
                                        lhsT=kTr[half:half + 64,
                                                 kb * P:(kb + 1) * P],
                                        rhs=qTr[half:half + 64, q0:q0 + 512],
                                        start=True,
                                        stop=True,
                                    )
                                if use_dve:
                                    nc.vector.tensor_scalar(
                                        bits2[:, e, :], s_ps[:, :], EA,
                                        EB if use_fix else EB_UNC,
                                        mult, add,
                                    )
                                else:
                                    nc.scalar.activation(
                                        p2[:, e, :], s_ps[:, :], Exp,
                                        scale=SCALE,
                                    )
                            if use_fix:
                                nc.vector._custom_dve(
                                    fix_op,
                                    out=p2[:, :, :],
                                    in0=bits2[:, :, :].bitcast(half_dt),
                                    in1=c3_sb[:, :],
                                    s0=MBITSF, s1=XC2, imm2=XC1,
                                )
                            # keep-mult: one op per k-block pair.
                            p_src = (
                                bits2[:, :, :].bitcast(half_dt)
                                if (use_dve and not use_fix) else p2[:, :, :]
                            )
                            pm2 = p_pool.tile(
                                [P, 2, QW], half_dt, tag="pm",
                                name=f"pm_{h}_{qh}_{kbp}",
                            )
                            mask_eng = (
                                nc.gpsimd if kbp in mask_gps else nc.vector
                            )
                            mask_eng.tensor_tensor(
                                pm2[:, :, :],
                                p_src,
                                keep_sb[:, 2 * kbp:2 * kbp + 2,
                                        qh * QW:(qh + 1) * QW],
                                mult,
                            )
                            for e in (0, 1):
                                kb = 2 * kbp + e
                                for qc in range(QW // 512):
                                    nc.tensor.matmul(
                                        ot[:, qc * 512:(qc + 1) * 512],
                                        lhsT=v_sb[:, kb, :],
                                        rhs=pm2[:, e, qc * 512:(qc + 1) * 512],
                                        start=(kb == 0),
                                        stop=(kb == KB - 1),
                                    )

                        ob_sb = ob_pool.tile(
                            [D + 1, QW], dt.float32, tag="ob",
                            name=f"ob_{h}_{qh}",
                        )
                        if ob_eng == "act":
                            nc.scalar.copy(ob_sb[:, :], ot[:, :])
                        else:
                            nc.vector.tensor_copy(ob_sb[:, :], ot[:, :])
                        nc.sync.dma_start(
                            out=out[h][:, qh * QW:(qh + 1) * QW],
                            in_=ob_sb[:, :],
                        )

            if repeats == 1:
                body(0)
            else:
                with tc.For_i(
                    0, repeats, 1,
                    hint_engines=(mybir.EngineType.PE, mybir.EngineType.DVE),
                ):
                    body(0)

    nc.compile()
    return nc


def get_nc(repeats=1, qk_dtype=QK_DTYPE, opts=()):
    key = ("nc", repeats, qk_dtype, frozenset(opts))
    if key not in _CACHE:
        _CACHE[key] = _build(repeats, qk_dtype, opts)
    return _CACHE[key]


def prep_in_maps(q, k, v, mask, qk_dtype=QK_DTYPE, half="float16"):
    bf16 = np.float16 if half == "float16" else ml_dtypes.bfloat16
    qk_np = np.float32 if qk_dtype == "float32r" else bf16
    q = np.asarray(q, dtype=np.float32).reshape(B * H, S, D)
    k = np.asarray(k, dtype=np.float32).reshape(B * H, S, D)
    vv = np.asarray(v, dtype=np.float32).reshape(B * H, S, D)
    mask = np.asarray(mask).reshape(S, S)
    keepT = np.ascontiguousarray((1 - mask).T.astype(np.float32)).astype(bf16)
    in_maps = []
    for c in range(N_CORES):
        sl = slice(c * HPC, (c + 1) * HPC)
        in_maps.append({
            "qT": np.ascontiguousarray(q[sl].transpose(0, 2, 1)).astype(qk_np),
            "kT": np.ascontiguousarray(k[sl].transpose(0, 2, 1)).astype(qk_np),
            "v": vv[sl].astype(bf16),
            "keepT": keepT,
        })
    return in_maps


def finish_output(core_results):
    """core_results: list of [HPC, D+1, S] fp32 arrays -> [B, H, S, D] fp32."""
    outs = []
    for r in core_results:
        r = np.asarray(r, dtype=np.float32)
        o = (r[:, :D, :] / r[:, D:D + 1, :]).transpose(0, 2, 1)
        outs.append(o)
    return np.concatenate(outs, axis=0).reshape(B, H, S, D).astype(np.float32)


def kernel(q, k, v, mask):
    from concourse import bass_utils

    nc = get_nc(1, opts=DEFAULT_OPTS)
    in_maps = prep_in_maps(q, k, v, mask)
    bkr = bass_utils.run_bass_kernel_spmd(nc, in_maps, list(range(N_CORES)))
    return finish_output([bkr.results[c]["out"] for c in range(N_CORES)])


# revision 16
# speedup vs baseline: 1.3070x; 1.3070x over previous
"""Masked multi-head attention (B=2, H=16, S=2048, D=64) on 8 TRN2 NeuronCores.

Sharding: batch*heads (32) split 4-heads-per-core across 8 cores; each core
computes full attention for its heads; the boolean mask is shared (broadcast
to every core). No cross-device communication.

Device algorithm (per head), in transposed "S^T" layout so softmax
probabilities land with the contraction (k) dim on partitions and feed the
P@V matmul with no on-device transposes:

  S^T[k, q]  = K^T[d, k].T @ Q^T[d, q]          (PE, d=64 contraction,
                                                 consecutive k-blocks packed
                                                 in opposite PE row halves)
  P^T[k, q]  = exp(scale * S^T) * keepT[k, q]
  O^T_aug    = V_aug[k, d+1].T @ P^T[k, q]      (PE, accumulated over k in
                                                 PSUM; V_aug has a ones
                                                 column -> row d holds the
                                                 softmax denominators)

The exp work is split across three engines to break the ACT bottleneck
(baseline: all 128 exps/core on ACT = ~140us busy vs ~110us PE):
  * ACT k-block pairs: exact spline exp (ACT), then keep-mult on DVE/GPSIMD.
  * DVE k-block pairs: 2-instruction Schraudolph in fp16 bit domain:
      op1 (DVE tensor_scalar): bits_i16 = int16(EA*s + EB)   [trunc+wrap;
          values stay in-range by construction, |s| <= ~60]
      op2 (custom DVE op):     p = bits.f16 * ((XC2*m + XC1)*m + XC0)
          where m = 1+frac extracted with BITWISE_AND/OR on the fp32
          bit pattern; max rel err 0.64% (vs 3% uncorrected sawtooth).
    then the same keep-mult.
The keep multiplies and the PSUM->SBUF output copies are distributed over
DVE / GPSIMD / ACT so every engine sits just under the PE roofline.

O^T_aug [65, 2048] fp32 is copied PSUM->SBUF and DMA'd to HBM; the final
normalize (divide by denominator row) + un-transpose happens on host.

Softmax needs no running-max: scores*scale ~ N(0,1), |max| < ~7, exp() is
safely in fp32/fp16-bitdomain range, and softmax is shift-invariant.
Masked entries match the reference exactly: reference adds -1e4 so exp
underflows to 0.0 in fp32; we multiply by keep=0.
"""

import sys

for _p in ("/opt/trn_rl_repo", "/root/.axon_site/_ro/trn_rl_repo"):
    if _p not in sys.path:
        sys.path.append(_p)

import numpy as np
import ml_dtypes

B, H, S, D = 2, 16, 2048, 64
N_CORES = 8
HPC = (B * H) // N_CORES  # heads per core
P = 128
KB = S // P               # k blocks per head
QH = 2                    # q halves (exp tile free dim = S/QH)
QW = S // QH
SCALE = 1.0 / 8.0         # 1/sqrt(D)

QK_DTYPE = "float32r"

# Schraudolph fp16-bitdomain exp constants (op1) + quadratic mantissa
# correction (op2).  Fit numerically for trunc() conversion, EB=15360:
# max rel err 0.64% on x in [-8, 8].
EA = float(1024.0 * SCALE / np.log(2.0))
EB = 15360.0          # for op2-corrected pairs
EB_UNC = 15312.0      # minimax-centered for uncorrected pairs (+-3%)
XC2, XC1, XC0 = 0.2263466, -0.66686962, 1.4347952
MBITSF = float(np.int32(0x007FE000).view(np.float32))
# vertex-form (Src1-free) correction: g = VA*(m+VX)^2 + 1, EB=EB2 (0.73% max)
EB2 = 15264.88
VX, VA = -1.48490674, 0.280136525

# --- engine assignment schedule -------------------------------------------
# Per (h, qh) iteration i = h*QH + qh (8 per core), per k-block-pair kbp
# (8 per iteration):
#   DVE_PAIRS[i % len]: kbps whose exp runs on DVE (op1 [+op2]); others ACT.
#   FIX_PAIRS[i % len]: DVE kbps that also get the op2 mantissa correction.
#   MASK_GPS[i % len]:  kbps whose keep-mult runs on GPSIMD (else DVE).
#   OB_ENGINE[i % len]: engine for the [65, QW] PSUM->SBUF output copy
#                       ("act" / "dve"; GPSIMD has no PSUM port).
# Measured HW per-op costs ([128, 2048] tile-pair): ACT exp-pair ~2.4us,
# DVE op1 ~2.5us, DVE op2 (custom) ~2.2-4.5us, DVE keep-mult ~1.1us,
# GPSIMD keep-mult ~5.4us, ob copy ~1.2us. PE ~112-118us total.
SCHEDULES = {
    # name: (DVE_PAIRS, FIX_PAIRS, MASK_GPS, OB_ENGINE)
    "v3": (
        (frozenset({2, 5}), frozenset({1, 6})),
        (frozenset(), frozenset()),
        (frozenset({3}), frozenset({7})),
        ("dve", "act"),
    ),
    "v3fix": (
        (frozenset({2, 5}), frozenset({1, 6})),
        (frozenset({2}), frozenset({6})),
        (frozenset({3}), frozenset({7})),
        ("dve", "act"),
    ),
    "v3allfix": (
        (frozenset({2, 5}), frozenset({1, 6})),
        (frozenset({2, 5}), frozenset({1, 6})),
        (frozenset({3}), frozenset({7})),
        ("dve", "act"),
    ),
    "v3g0": (
        (frozenset({2, 5}), frozenset({1, 6})),
        (frozenset(), frozenset()),
        (frozenset(), frozenset()),
        ("dve", "act"),
    ),
    "all_act": (
        (frozenset(), frozenset()),
        (frozenset(), frozenset()),
        (frozenset({3}), frozenset({7})),
        ("dve", "act"),
    ),
}

DEFAULT_OPTS = ("v3fix",)

_CACHE = {}


def _register_schraud_fix():
    """Runtime-register the custom DVE mantissa-correction op.

    out = in0 * ((s1*m + imm2)*m + c3),  m = or(and(bits(in0), s0), 1.0)
    (s0 = fp32 mantissa mask 0x007FE000 as float; c3 via in1 latch).
    """
    from concourse import dve_ops
    from concourse.dve_spec import (
        Spec, Src0, C0, C1, C2, C3, One, Bin, AluOp, lower, _spill_c3_to_src1,
    )
    from concourse.dve_uop import DveOpSpec

    name = "SCHRAUD_FIX_ANT"
    for o in dve_ops.OPS:
        if o.name == name:
            return o

    m = Bin(AluOp.BITWISE_OR, Bin(AluOp.BITWISE_AND, Src0, C0), One)
    g = (C1 * m + C2) * m + C3
    body = _spill_c3_to_src1(Src0 * g)

    def ref(in0, in1, s0, s1, imm2):
        x32 = in0.astype(np.float32)
        bits = x32.view(np.uint32)
        maskb = np.float32(
            s0 if np.isscalar(s0) else np.asarray(s0).ravel()[0]
        ).view(np.uint32)
        mbits = (bits & maskb) | np.float32(1.0).view(np.uint32)
        mm = mbits.view(np.float32)
        c3 = np.float32(np.asarray(in1).ravel()[0])
        gg = (np.float32(s1) * mm + np.float32(imm2)) * mm + c3
        return x32 * gg

    op = dve_ops.DveOp(name, Spec(body=body, reference=ref), subdim=False,
                       uops_sha={})
    _install_dve_op(op, rd1_en=True)
    return op


def _install_dve_op(op, rd1_en):
    from concourse import dve_ops
    from concourse.dve_spec import lower
    from concourse.dve_uop import DveOpSpec
    from concourse.dve_ops import get_dve_sub_opcode

    dve_ops.OPS.append(op)
    dve_ops.CUSTOM_DVE_SPECS[op.name] = op.spec
    dve_ops._SUB_OPCODE_FOR_NAME[op.name] = (
        max(dve_ops._SUB_OPCODE_FOR_NAME.values()) + 1
    )
    assert dve_ops._SUB_OPCODE_FOR_NAME[op.name] < 0x20
    for ver in ("v3", "v4"):
        uops = lower(op.spec, ver=ver)
        tmp = DveOpSpec(name=op.name, opcode=get_dve_sub_opcode(op.name),
                        uops=uops, rd1_en=rd1_en)
        op.uops_sha[ver] = tmp.sha(ver)


def _register_schraud_fix2():
    """Src1-free vertex-form correction: out = in0 * (s1*(m+s0)^2 + 1)."""
    from concourse import dve_ops
    from concourse.dve_spec import Spec, Src0, C0, C1, C2, One, Bin, AluOp, sq

    name = "SCHRAUD_FIX2_ANT"
    for o in dve_ops.OPS:
        if o.name == name:
            return o

    m = Bin(AluOp.BITWISE_OR, Bin(AluOp.BITWISE_AND, Src0, C2), One)
    g = C1 * sq(m + C0) + One
    body = Src0 * g

    def ref(in0, in1, s0, s1, imm2):
        x32 = in0.astype(np.float32)
        bits = x32.view(np.uint32)
        maskb = np.float32(imm2).view(np.uint32)
        mbits = (bits & maskb) | np.float32(1.0).view(np.uint32)
        mm = mbits.view(np.float32)
        s0v = np.float32(s0 if np.isscalar(s0) else np.asarray(s0).ravel()[0])
        s1v = np.float32(s1 if np.isscalar(s1) else np.asarray(s1).ravel()[0])
        gg = s1v * (mm + s0v) ** 2 + np.float32(1.0)
        return x32 * gg

    op = dve_ops.DveOp(name, Spec(body=body, reference=ref), subdim=False,
                       uops_sha={})
    _install_dve_op(op, rd1_en=False)
    return op


def _build(repeats=1, qk_dtype=QK_DTYPE, opts=()):
    opts = frozenset(opts)
    import concourse.mybir as mybir
    import concourse.tile as tile
    from concourse import bacc

    use_fix2 = "fix2" in opts
    fix_op = _register_schraud_fix2() if use_fix2 else _register_schraud_fix()
    eb_fix = EB2 if use_fix2 else EB

    dt = mybir.dt
    qk_dt = getattr(dt, qk_dtype)
    half_dt = dt.float16
    nc = bacc.Bacc(
        "TRN2", target_bir_lowering=False, debug=False, num_devices=N_CORES
    )

    qT = nc.dram_tensor("qT", [HPC, D, S], qk_dt, kind="ExternalInput").ap()
    kT = nc.dram_tensor("kT", [HPC, D, S], qk_dt, kind="ExternalInput").ap()
    v = nc.dram_tensor("v", [HPC, S, D], half_dt, kind="ExternalInput").ap()
    keepT = nc.dram_tensor("keepT", [S, S], half_dt, kind="ExternalInput").ap()
    out = nc.dram_tensor(
        "out", [HPC, D + 1, S], dt.float32, kind="ExternalOutput"
    ).ap()

    Exp = mybir.ActivationFunctionType.Exp
    mult = mybir.AluOpType.mult
    add = mybir.AluOpType.add

    sched_name = next((o for o in opts if o in SCHEDULES), "v3fix")
    DVE_PAIRS, FIX_PAIRS, MASK_GPS, OB_ENGINE = SCHEDULES[sched_name]

    with tile.TileContext(nc) as tc:
        with (
            tc.tile_pool(name="keep_pool", bufs=1) as keep_pool,
            tc.tile_pool(name="const_pool", bufs=1) as const_pool,
            tc.tile_pool(name="qk_pool", bufs=2) as qk_pool,
            tc.tile_pool(name="v_pool", bufs=2) as v_pool,
            tc.tile_pool(name="p_pool", bufs=3) as p_pool,
            tc.tile_pool(name="ob_pool", bufs=2) as ob_pool,
            tc.tile_pool(name="s_psum", bufs=3, space="PSUM") as s_psum,
            tc.tile_pool(name="o_psum", bufs=1, space="PSUM") as o_psum,
        ):
            keep_sb = keep_pool.tile([P, KB, S], half_dt)
            for kb in range(KB):
                nc.sync.dma_start(
                    out=keep_sb[:, kb, :], in_=keepT[kb * P:(kb + 1) * P, :]
                )
            c3_sb = const_pool.tile([P, 1], dt.float32)
            nc.gpsimd.memset(c3_sb[:, :], XC0)

            def body(rep):
                for h in range(HPC):
                    # Q^T/K^T [64, S] duplicated into both partition halves
                    # so consecutive k-blocks run in opposite PE row halves.
                    qTr = qk_pool.tile([P, S], qk_dt, tag="qTr", name=f"qTr_{h}")
                    kTr = qk_pool.tile([P, S], qk_dt, tag="kTr", name=f"kTr_{h}")
                    for half in (0, 1):
                        nc.sync.dma_start(
                            out=qTr[half * 64:(half + 1) * 64, :], in_=qT[h]
                        )
                        nc.sync.dma_start(
                            out=kTr[half * 64:(half + 1) * 64, :], in_=kT[h]
                        )

                    # V with an appended ones column (denominator column).
                    v_sb = v_pool.tile(
                        [P, KB, D + 1], half_dt, tag="v", name=f"v_{h}"
                    )
                    v_re = v[h].rearrange("(kb p) d -> p kb d", p=P)
                    nc.sync.dma_start(out=v_sb[:, :, 0:D], in_=v_re)
                    nc.gpsimd.memset(v_sb[:, :, D:D + 1], 1.0)

                    for qh in range(QH):
                        it = h * QH + qh
                        dve_pairs = DVE_PAIRS[it % len(DVE_PAIRS)]
                        fix_pairs = FIX_PAIRS[it % len(FIX_PAIRS)]
                        mask_gps = MASK_GPS[it % len(MASK_GPS)]
                        ob_eng = OB_ENGINE[it % len(OB_ENGINE)]

                        ot = o_psum.tile(
                            [D + 1, QW], dt.float32, tag="ot",
                            name=f"ot_{h}_{qh}",
                        )
                        for kbp in range(KB // 2):
                            use_dve = kbp in dve_pairs
                            use_fix = use_dve and kbp in fix_pairs
                            if not (use_dve and not use_fix):
                                p2 = p_pool.tile(
                                    [P, 2, QW], half_dt, tag="p",
                                    name=f"p_{h}_{qh}_{kbp}",
                                )
                            if use_dve:
                                bits2 = p_pool.tile(
                                    [P, 2, QW], dt.int16, tag="bits",
                                    name=f"bits_{h}_{qh}_{kbp}",
                                )
                            for e in (0, 1):
                                kb = 2 * kbp + e
                                half = 64 * e
                                s_ps = s_psum.tile(
                                    [P, QW], dt.float32, tag="s",
                                    name=f"s_{h}_{qh}_{kb}",
                                )
                                for qc in range(QW // 512):
                                    q0 = qh * QW + qc * 512
                                    nc.tensor.matmul(
                                        s_ps[:, qc * 512:(qc + 1) * 512],
                                        lhsT=kTr[half:half + 64,
                                                 kb * P:(kb + 1) * P],
                                        rhs=qTr[half:half + 64, q0:q0 + 512],
                                        start=True,
                                        stop=True,
                                    )
                                if use_dve:
                                    nc.vector.tensor_scalar(
                                        bits2[:, e, :], s_ps[:, :], EA,
                                        eb_fix if use_fix else EB_UNC,
                                        mult, add,
                                    )
                                else:
                                    nc.scalar.activation(
                                        p2[:, e, :], s_ps[:, :], Exp,
                                        scale=SCALE,
                                    )
                            if use_fix:
                                if use_fix2:
                                    nc.vector._custom_dve(
                                        fix_op,
                                        out=p2[:, :, :],
                                        in0=bits2[:, :, :].bitcast(half_dt),
                                        s0=VX, s1=VA, imm2=MBITSF,
                                    )
                                else:
                                    nc.vector._custom_dve(
                                        fix_op,
                                        out=p2[:, :, :],
                                        in0=bits2[:, :, :].bitcast(half_dt),
                                        in1=c3_sb[:, :],
                                        s0=MBITSF, s1=XC2, imm2=XC1,
                                    )
                            # keep-mult: one op per k-block pair.
                            p_src = (
                                bits2[:, :, :].bitcast(half_dt)
                                if (use_dve and not use_fix) else p2[:, :, :]
                            )
                            pm2 = p_pool.tile(
                                [P, 2, QW], half_dt, tag="pm",
                                name=f"pm_{h}_{qh}_{kbp}",
                            )
                            mask_eng = (
                                nc.gpsimd if kbp in mask_gps else nc.vector
                            )
                            mask_eng.tensor_tensor(
                                pm2[:, :, :],
                                p_src,
                                keep_sb[:, 2 * kbp:2 * kbp + 2,
                                        qh * QW:(qh + 1) * QW],
                                mult,
                            )
                            for e in (0, 1):
                                kb = 2 * kbp + e
                                for qc in range(QW // 512):
                                    nc.tensor.matmul(
                                        ot[:, qc * 512:(qc + 1) * 512],
                                        lhsT=v_sb[:, kb, :],
                                        rhs=pm2[:, e, qc * 512:(qc + 1) * 512],
                                        start=(kb == 0),
                                        stop=(kb == KB - 1),
                                    )

                        ob_sb = ob_pool.tile(
                            [D + 1, QW], dt.float32, tag="ob",
                            name=f"ob_{h}_{qh}",
                        )
                        if ob_eng == "act":
                            nc.scalar.copy(ob_sb[:, :], ot[:, :])
                        else:
                            nc.vector.tensor_copy(ob_sb[:, :], ot[:, :])
                        nc.sync.dma_start(
                            out=out[h][:, qh * QW:(qh + 1) * QW],
                            in_=ob_sb[:, :],
                        )

            if repeats == 1:
                body(0)
            else:
                with tc.For_i(
                    0, repeats, 1,
                    hint_engines=(mybir.EngineType.PE, mybir.EngineType.DVE),
                ):
                    body(0)

    nc.compile()
    return nc


def get_nc(repeats=1, qk_dtype=QK_DTYPE, opts=()):
    key = ("nc", repeats, qk_dtype, frozenset(opts))
    if key not in _CACHE:
        _CACHE[key] = _build(repeats, qk_dtype, opts)
    return _CACHE[key]


def prep_in_maps(q, k, v, mask, qk_dtype=QK_DTYPE, half="float16"):
    bf16 = np.float16 if half == "float16" else ml_dtypes.bfloat16
    qk_np = np.float32 if qk_dtype == "float32r" else bf16
    q = np.asarray(q, dtype=np.float32).reshape(B * H, S, D)
    k = np.asarray(k, dtype=np.float32).reshape(B * H, S, D)
    vv = np.asarray(v, dtype=np.float32).reshape(B * H, S, D)
    mask = np.asarray(mask).reshape(S, S)
    keepT = np.ascontiguousarray((1 - mask).T.astype(np.float32)).astype(bf16)
    in_maps = []
    for c in range(N_CORES):
        sl = slice(c * HPC, (c + 1) * HPC)
        in_maps.append({
            "qT": np.ascontiguousarray(q[sl].transpose(0, 2, 1)).astype(qk_np),
            "kT": np.ascontiguousarray(k[sl].transpose(0, 2, 1)).astype(qk_np),
            "v": vv[sl].astype(bf16),
            "keepT": keepT,
        })
    return in_maps


def finish_output(core_results):
    """core_results: list of [HPC, D+1, S] fp32 arrays -> [B, H, S, D] fp32."""
    outs = []
    for r in core_results:
        r = np.asarray(r, dtype=np.float32)
        o = (r[:, :D, :] / r[:, D:D + 1, :]).transpose(0, 2, 1)
        outs.append(o)
    return np.concatenate(outs, axis=0).reshape(B, H, S, D).astype(np.float32)


def kernel(q, k, v, mask):
    from concourse import bass_utils

    nc = get_nc(1, opts=DEFAULT_OPTS)
    in_maps = prep_in_maps(q, k, v, mask)
    bkr = bass_utils.run_bass_kernel_spmd(nc, in_maps, list(range(N_CORES)))
    return finish_output([bkr.results[c]["out"] for c in range(N_CORES)])
